# revision 15
# baseline (speedup 1.0000x reference)
"""LoRALinear Trainium2 kernel.

out = x @ W^T + bias + 2.0 * ((x @ A^T) @ B^T)

Strategy:
  - 2D sharding over 8 NeuronCores: M (token) dim split 2-way, out_features
    split 4-way. Each core: x-shard [4096, 4096], W-shard [1024, 4096].
  - Host ships k-major (pre-transposed) layouts of x, W and B — pure data
    layout, all arithmetic stays on device.
  - LoRA is folded into the cached weights on-chip: WeffT = W^T + (2*B@A)^T,
    via 16-partition matmuls (A is naturally [r, k] so it is the stationary
    operand with no transpose needed).
  - fp16 compute: f32->f16 cast during DMA (SWDGE), fp16 matmuls (1 cycle/row)
    accumulating in fp32 PSUM, bias added on PSUM->SBUF evict (DVE).
  - Phase 0 (W streaming) keeps the PE dense: 3 m-blocks k-outer-interleaved
    with the fold (6 PSUM banks + 2 fold banks), a 12-deep W staging pool so
    the W stream never waits on compute, and small filler matmuls per k-tile
    so the HAM clock gate stays at 2.4 GHz throughout.
"""

import numpy as np

IN_F = 4096
OUT_F = 4096
R = 16
SCALING = 2.0
M = 4 * 2048  # 8192 tokens

N_CORES = 8
M_SPLIT = 2
O_SPLIT = 4
M_SH = M // M_SPLIT      # 4096 rows per core
O_SH = OUT_F // O_SPLIT  # 1024 out-features per core
K = IN_F
KT = K // 128            # 32 k-tiles
MG = M_SH // 512         # 8 m-groups of 512 rows
NPH0 = 3                 # m-blocks interleaved into the W-stream phase

_NC_CACHE = {}
LAST_RESULT = None


def _build():
    import concourse.mybir as mybir
    import concourse.tile as tile
    from concourse import bacc

    f32, f16 = mybir.dt.float32, mybir.dt.float16

    nc = bacc.Bacc(
        "TRN2", target_bir_lowering=False, debug=False, num_devices=N_CORES
    )
    xt_d = nc.dram_tensor("xt", [K, M_SH], f32, kind="ExternalInput")
    wt_d = nc.dram_tensor("wt", [K, O_SH], f32, kind="ExternalInput")
    bt_d = nc.dram_tensor("bt", [R, O_SH], f32, kind="ExternalInput")
    bias_d = nc.dram_tensor("bias", [O_SH], f32, kind="ExternalInput")
    a_d = nc.dram_tensor("a", [R, K], f32, kind="ExternalInput")
    out_d = nc.dram_tensor("out", [M_SH, O_SH], f32, kind="ExternalOutput")

    with tile.TileContext(nc) as tc:
        with (
            tc.tile_pool(name="const", bufs=1) as const,
            tc.tile_pool(name="wtp", bufs=KT) as wtp,
            tc.tile_pool(name="wraw", bufs=8) as wraw,
            tc.tile_pool(name="xtp", bufs=KT + 8) as xtp,
            tc.tile_pool(name="outp", bufs=4) as outp,
        ):
            wt_tiles = [None] * KT
            with (
                tc.tile_pool(name="psum_ba", bufs=2, space="PSUM") as psum_ba,
                tc.tile_pool(name="psum_m", bufs=2 * NPH0, space="PSUM") as psum_m,
            ):
                # Warmup burst first in DVE+PE program order: junk memset
                # as soon as the DVE comes up, then dependency-free matmuls
                # flip the PE HAM clock gate (1.2 -> 2.4 GHz) and bridge
                # until the first W/x tiles land.
                junk = const.tile([128, 512], f16)
                nc.vector.memset(junk[:], 0.0)
                pwarm = psum_ba.tile([128, 512], f32, name="ba")
                for _ in range(20):
                    nc.tensor.matmul(
                        pwarm[:], junk[:, 0:128], junk[:], start=True, stop=True
                    )

                # ---- constants ----
                bias_bc = const.tile([128, O_SH], f32)
                a_sb = const.tile([R, K], f16)
                nc.gpsimd.dma_start(a_sb[:], a_d[:])
                btr = const.tile([R, O_SH], f16)
                nc.gpsimd.dma_start(btr[:], bt_d[:])
                bt_sb = const.tile([R, O_SH], f16)  # 2 * B^T
                nc.vector.tensor_scalar_mul(bt_sb[:], btr[:], SCALING)
                nc.sync.dma_start(
                    bias_bc[:], bias_d[:].partition_broadcast(128)
                )

                def evict(g, mb, p0, p1):
                    osb = outp.tile([128, O_SH], f32, name="osb")
                    nc.vector.tensor_add(osb[:, 0:512], p0[:], bias_bc[:, 0:512])
                    nc.vector.tensor_add(
                        osb[:, 512:1024], p1[:], bias_bc[:, 512:1024]
                    )
                    nc.sync.dma_start(
                        out_d[g * 512 + mb * 128 : g * 512 + (mb + 1) * 128, :],
                        osb[:],
                    )

                def mblock_mms(pcol, xts, kt, mb):
                    st, sp = kt == 0, kt == KT - 1
                    ms = slice(mb * 128, (mb + 1) * 128)
                    nc.tensor.matmul(
                        pcol[0][:], xts[kt][:, ms], wt_tiles[kt][:, 0:512],
                        start=st, stop=sp,
                    )
                    nc.tensor.matmul(
                        pcol[1][:], xts[kt][:, ms], wt_tiles[kt][:, 512:1024],
                        start=st, stop=sp,
                    )

                # ---- Phase 0: W and x(g0) stream interleaved on the sync
                # HWDGE queue (strict FIFO -> lockstep arrival at HBM rate),
                # x(g0) staged f32 and cast to f16 on the DVE. The fold for
                # k+1 issues before the main matmuls for k so the PE queue
                # head never sits on a fresh semaphore wait. ----
                PRE = 4  # DMA pipeline depth ahead of the fold
                wt_raws = [None] * KT
                xs_tiles = [None] * KT
                xts0 = [None] * KT

                def wx_dma(kt):
                    ks = slice(kt * 128, (kt + 1) * 128)
                    w_raw = wraw.tile([128, O_SH], f32, name="w_raw")
                    nc.sync.dma_start(w_raw[:], wt_d[ks, :])
                    xs = wraw.tile([128, 512], f32, name="xs")
                    nc.sync.dma_start(xs[:], xt_d[ks, 0:512])
                    wt_raws[kt] = w_raw
                    xs_tiles[kt] = xs

                def xcast(kt):
                    x_t = xtp.tile([128, 512], f16, name="x_t")
                    nc.vector.tensor_copy(x_t[:], xs_tiles[kt][:])
                    xts0[kt] = x_t

                def fold(kt):
                    ks = slice(kt * 128, (kt + 1) * 128)
                    # Filler MMs: absorb the DVE->PE fold-bank recycle
                    # wait so the HAM busy-window never sees an idle PE
                    # during the W stream.
                    jt = psum_ba.tile([128, 512], f32, name="ba")
                    for j in range(2):
                        nc.tensor.matmul(
                            jt[:, j * 128 : (j + 1) * 128],
                            junk[:, 0:128], junk[:, 0:128],
                            start=True, stop=True,
                        )
                    pba = psum_ba.tile([128, 512], f32, name="ba")
                    pbb = psum_ba.tile([128, 512], f32, name="ba")
                    wt_t = wtp.tile([128, O_SH], f16, name="wt_t")
                    nc.tensor.matmul(
                        pba[:], a_sb[:, ks], bt_sb[:, 0:512],
                        start=True, stop=True,
                    )
                    nc.tensor.matmul(
                        pbb[:], a_sb[:, ks], bt_sb[:, 512:1024],
                        start=True, stop=True,
                    )
                    nc.vector.tensor_add(
                        wt_t[:, 0:512], pba[:], wt_raws[kt][:, 0:512]
                    )
                    nc.vector.tensor_add(
                        wt_t[:, 512:1024], pbb[:], wt_raws[kt][:, 512:1024]
                    )
                    wt_tiles[kt] = wt_t

                for kt in range(PRE):
                    wx_dma(kt)
                for kt in range(PRE):
                    xcast(kt)
                fold(0)
                pcols = [
                    [psum_m.tile([128, 512], f32, name="pm") for h in (0, 1)]
                    for _ in range(NPH0)
                ]
                for kt in range(KT):
                    if kt + PRE < KT:
                        wx_dma(kt + PRE)
                    if kt + 1 < KT:
                        fold(kt + 1)
                    if kt + PRE < KT:
                        xcast(kt + PRE)
                    for mb in range(NPH0):
                        mblock_mms(pcols[mb], xts0, kt, mb)
                for mb in range(NPH0):
                    evict(0, mb, pcols[mb][0], pcols[mb][1])

                # ---- remaining m-blocks: g0's tail, then groups 1..MG-1
                # with x streamed f32->f16 on the gpsimd SWDGE ----
                x_groups = [xts0]
                for g in range(1, MG):
                    gs = slice(g * 512, (g + 1) * 512)
                    xts = []
                    for kt in range(KT):
                        x_t = xtp.tile([128, 512], f16, name="x_t")
                        nc.gpsimd.dma_start(
                            x_t[:], xt_d[kt * 128 : (kt + 1) * 128, gs]
                        )
                        xts.append(x_t)
                    x_groups.append(xts)

                for g in range(MG):
                    xts = x_groups[g]
                    remaining = range(NPH0, 4) if g == 0 else range(4)
                    for mb in remaining:
                        p0 = psum_m.tile([128, 512], f32, name="pm")
                        p1 = psum_m.tile([128, 512], f32, name="pm")
                        for kt in range(KT):
                            mblock_mms((p0, p1), xts, kt, mb)
                        evict(g, mb, p0, p1)

    nc.compile()
    return nc


def _get_nc():
    if "nc" not in _NC_CACHE:
        _NC_CACHE["nc"] = _build()
    return _NC_CACHE["nc"]


def kernel(x, weight, bias, A, B):
    global LAST_RESULT
    from concourse.bass_utils import run_bass_kernel_spmd

    x = np.asarray(x, dtype=np.float32).reshape(M, K)
    weight = np.asarray(weight, dtype=np.float32)
    bias = np.asarray(bias, dtype=np.float32)
    A = np.ascontiguousarray(np.asarray(A, dtype=np.float32))
    B = np.asarray(B, dtype=np.float32)

    # Host-side layout prep (transposes only; no arithmetic).
    xt_halves = [
        np.ascontiguousarray(x[mi * M_SH : (mi + 1) * M_SH].T)
        for mi in range(M_SPLIT)
    ]
    wt_quads = []
    bt_quads = []
    bias_quads = []
    for oi in range(O_SPLIT):
        os_ = slice(oi * O_SH, (oi + 1) * O_SH)
        wt_quads.append(np.ascontiguousarray(weight[os_].T))
        bt_quads.append(np.ascontiguousarray(B[os_].T))
        bias_quads.append(np.ascontiguousarray(bias[os_]))

    nc = _get_nc()
    in_maps = []
    for c in range(N_CORES):
        mi, oi = divmod(c, O_SPLIT)
        in_maps.append(
            {
                "xt": xt_halves[mi],
                "wt": wt_quads[oi],
                "bt": bt_quads[oi],
                "bias": bias_quads[oi],
                "a": A,
            }
        )

    res = run_bass_kernel_spmd(nc, in_maps, list(range(N_CORES)))
    LAST_RESULT = res

    out = np.empty((M, OUT_F), np.float32)
    for c in range(N_CORES):
        mi, oi = divmod(c, O_SPLIT)
        out[mi * M_SH : (mi + 1) * M_SH, oi * O_SH : (oi + 1) * O_SH] = (
            res.results[c]["out"]
        )
    return out.reshape(4, 2048, OUT_F)


# revision 17
# speedup vs baseline: 1.1406x; 1.1406x over previous
"""LoRALinear Trainium2 kernel.

out = x @ W^T + bias + 2.0 * ((x @ A^T) @ B^T)

Strategy:
  - 2D sharding over 8 NeuronCores: M (token) dim split 2-way, out_features
    split 4-way. Each core: x-shard [4096, 4096], W-shard [1024, 4096].
  - Host ships k-major (pre-transposed) layouts of x, W and B — pure data
    layout, all arithmetic stays on device.
  - LoRA is folded into the cached weights on-chip: WeffT = W^T + (2*B@A)^T,
    via 16-partition matmuls (A is naturally [r, k] so it is the stationary
    operand with no transpose needed).
  - fp16 compute: f32->f16 cast during DMA (SWDGE), fp16 matmuls (1 cycle/row)
    accumulating in fp32 PSUM, bias added on PSUM->SBUF evict (DVE).
  - Phase 0 (W streaming) keeps the PE dense: 3 m-blocks k-outer-interleaved
    with the fold (6 PSUM banks + 2 fold banks), a 12-deep W staging pool so
    the W stream never waits on compute, and small filler matmuls per k-tile
    so the HAM clock gate stays at 2.4 GHz throughout.
"""

import numpy as np

IN_F = 4096
OUT_F = 4096
R = 16
SCALING = 2.0
M = 4 * 2048  # 8192 tokens

N_CORES = 8
M_SPLIT = 2
O_SPLIT = 4
M_SH = M // M_SPLIT      # 4096 rows per core
O_SH = OUT_F // O_SPLIT  # 1024 out-features per core
K = IN_F
KT = K // 128            # 32 k-tiles
MG = M_SH // 512         # 8 m-groups of 512 rows
NPH0 = 3                 # m-blocks interleaved into the W-stream phase

_NC_CACHE = {}
LAST_RESULT = None


def _build():
    import concourse.mybir as mybir
    import concourse.tile as tile
    from concourse import bacc

    f32, f16 = mybir.dt.float32, mybir.dt.float16

    nc = bacc.Bacc(
        "TRN2", target_bir_lowering=False, debug=False, num_devices=N_CORES
    )
    xt_d = nc.dram_tensor("xt", [K, M_SH], f32, kind="ExternalInput")
    wt_d = nc.dram_tensor("wt", [K, O_SH], f32, kind="ExternalInput")
    bt_d = nc.dram_tensor("bt", [R, O_SH], f32, kind="ExternalInput")
    bias_d = nc.dram_tensor("bias", [O_SH], f32, kind="ExternalInput")
    a_d = nc.dram_tensor("a", [R, K], f32, kind="ExternalInput")
    out_d = nc.dram_tensor("out", [M_SH, O_SH], f32, kind="ExternalOutput")

    with tile.TileContext(nc) as tc:
        with (
            tc.tile_pool(name="const", bufs=1) as const,
            tc.tile_pool(name="wtp", bufs=KT) as wtp,
            tc.tile_pool(name="wraw", bufs=8) as wraw,
            tc.tile_pool(name="xtp", bufs=KT + 16) as xtp,
            tc.tile_pool(name="outp", bufs=4) as outp,
        ):
            wt_tiles = [None] * KT
            with (
                tc.tile_pool(name="psum_ba", bufs=2, space="PSUM") as psum_ba,
                tc.tile_pool(name="psum_m", bufs=2 * NPH0, space="PSUM") as psum_m,
            ):
                # Warmup burst first in DVE+PE program order: junk memset
                # as soon as the DVE comes up, then dependency-free matmuls
                # flip the PE HAM clock gate (1.2 -> 2.4 GHz) and bridge
                # until the first W/x tiles land.
                junk = const.tile([128, 512], f16)
                nc.vector.memset(junk[:], 0.0)
                pwarm = psum_ba.tile([128, 512], f32, name="ba")
                for _ in range(20):
                    nc.tensor.matmul(
                        pwarm[:], junk[:, 0:128], junk[:], start=True, stop=True
                    )

                # ---- constants ----
                bias_bc = const.tile([128, O_SH], f32)
                a_sb = const.tile([R, K], f16)
                nc.gpsimd.dma_start(a_sb[:], a_d[:])
                btr = const.tile([R, O_SH], f16)
                nc.gpsimd.dma_start(btr[:], bt_d[:])
                bt_sb = const.tile([R, O_SH], f16)  # 2 * B^T
                nc.vector.tensor_scalar_mul(bt_sb[:], btr[:], SCALING)
                nc.sync.dma_start(
                    bias_bc[:], bias_d[:].partition_broadcast(128)
                )

                def evict(g, mb, p0, p1):
                    osb = outp.tile([128, O_SH], f32, name="osb")
                    nc.vector.tensor_add(osb[:, 0:512], p0[:], bias_bc[:, 0:512])
                    nc.vector.tensor_add(
                        osb[:, 512:1024], p1[:], bias_bc[:, 512:1024]
                    )
                    nc.sync.dma_start(
                        out_d[g * 512 + mb * 128 : g * 512 + (mb + 1) * 128, :],
                        osb[:],
                    )

                def mblock_mms(pcol, xts, kt, mb):
                    st, sp = kt == 0, kt == KT - 1
                    ms = slice(mb * 128, (mb + 1) * 128)
                    nc.tensor.matmul(
                        pcol[0][:], xts[kt][:, ms], wt_tiles[kt][:, 0:512],
                        start=st, stop=sp,
                    )
                    nc.tensor.matmul(
                        pcol[1][:], xts[kt][:, ms], wt_tiles[kt][:, 512:1024],
                        start=st, stop=sp,
                    )

                # ---- Phase 0: W and x(g0) stream interleaved on the sync
                # HWDGE queue (strict FIFO -> lockstep arrival at HBM rate),
                # x(g0) staged f32 and cast to f16 on the DVE. The fold for
                # k+1 issues before the main matmuls for k so the PE queue
                # head never sits on a fresh semaphore wait. ----
                PRE = 4  # DMA pipeline depth ahead of the fold
                wt_raws = [None] * KT
                xs_tiles = [None] * KT
                xts0 = [None] * KT

                def wx_dma(kt):
                    ks = slice(kt * 128, (kt + 1) * 128)
                    w_raw = wraw.tile([128, O_SH], f32, name="w_raw")
                    nc.sync.dma_start(w_raw[:], wt_d[ks, :])
                    xs = wraw.tile([128, 512], f32, name="xs")
                    nc.sync.dma_start(xs[:], xt_d[ks, 0:512])
                    wt_raws[kt] = w_raw
                    xs_tiles[kt] = xs

                def xcast(kt):
                    # ScalarE (idle otherwise) does the f32->f16 cast so the
                    # DVE only carries the fold adds in phase 0.
                    x_t = xtp.tile([128, 512], f16, name="x_t")
                    nc.scalar.copy(x_t[:], xs_tiles[kt][:])
                    xts0[kt] = x_t

                def fold(kt):
                    ks = slice(kt * 128, (kt + 1) * 128)
                    # Filler MMs: absorb the DVE->PE fold-bank recycle
                    # wait so the HAM busy-window never sees an idle PE
                    # during the W stream.
                    jt = psum_ba.tile([128, 512], f32, name="ba")
                    for j in range(2):
                        nc.tensor.matmul(
                            jt[:, j * 128 : (j + 1) * 128],
                            junk[:, 0:128], junk[:, 0:128],
                            start=True, stop=True,
                        )
                    pba = psum_ba.tile([128, 512], f32, name="ba")
                    pbb = psum_ba.tile([128, 512], f32, name="ba")
                    wt_t = wtp.tile([128, O_SH], f16, name="wt_t")
                    nc.tensor.matmul(
                        pba[:], a_sb[:, ks], bt_sb[:, 0:512],
                        start=True, stop=True,
                    )
                    nc.tensor.matmul(
                        pbb[:], a_sb[:, ks], bt_sb[:, 512:1024],
                        start=True, stop=True,
                    )
                    nc.vector.tensor_add(
                        wt_t[:, 0:512], pba[:], wt_raws[kt][:, 0:512]
                    )
                    nc.vector.tensor_add(
                        wt_t[:, 512:1024], pbb[:], wt_raws[kt][:, 512:1024]
                    )
                    wt_tiles[kt] = wt_t

                for kt in range(PRE):
                    wx_dma(kt)
                for kt in range(PRE):
                    xcast(kt)
                fold(0)
                pcols = [
                    [psum_m.tile([128, 512], f32, name="pm") for h in (0, 1)]
                    for _ in range(NPH0)
                ]
                for kt in range(KT):
                    if kt + PRE < KT:
                        wx_dma(kt + PRE)
                    if kt + 1 < KT:
                        fold(kt + 1)
                    if kt + PRE < KT:
                        xcast(kt + PRE)
                    for mb in range(NPH0):
                        mblock_mms(pcols[mb], xts0, kt, mb)
                for mb in range(NPH0):
                    evict(0, mb, pcols[mb][0], pcols[mb][1])

                # ---- remaining m-blocks: g0's tail, then groups 1..MG-1
                # with x streamed f32->f16 on the gpsimd SWDGE ----
                x_groups = [xts0]
                for g in range(1, MG):
                    gs = slice(g * 512, (g + 1) * 512)
                    xts = []
                    for kt in range(KT):
                        x_t = xtp.tile([128, 512], f16, name="x_t")
                        nc.gpsimd.dma_start(
                            x_t[:], xt_d[kt * 128 : (kt + 1) * 128, gs]
                        )
                        xts.append(x_t)
                    x_groups.append(xts)

                for g in range(MG):
                    xts = x_groups[g]
                    remaining = range(NPH0, 4) if g == 0 else range(4)
                    for mb in remaining:
                        p0 = psum_m.tile([128, 512], f32, name="pm")
                        p1 = psum_m.tile([128, 512], f32, name="pm")
                        for kt in range(KT):
                            mblock_mms((p0, p1), xts, kt, mb)
                        evict(g, mb, p0, p1)

    nc.compile()
    return nc


def _get_nc():
    if "nc" not in _NC_CACHE:
        _NC_CACHE["nc"] = _build()
    return _NC_CACHE["nc"]


def kernel(x, weight, bias, A, B):
    global LAST_RESULT
    from concourse.bass_utils import run_bass_kernel_spmd

    x = np.asarray(x, dtype=np.float32).reshape(M, K)
    weight = np.asarray(weight, dtype=np.float32)
    bias = np.asarray(bias, dtype=np.float32)
    A = np.ascontiguousarray(np.asarray(A, dtype=np.float32))
    B = np.asarray(B, dtype=np.float32)

    # Host-side layout prep (transposes only; no arithmetic).
    xt_halves = [
        np.ascontiguousarray(x[mi * M_SH : (mi + 1) * M_SH].T)
        for mi in range(M_SPLIT)
    ]
    wt_quads = []
    bt_quads = []
    bias_quads = []
    for oi in range(O_SPLIT):
        os_ = slice(oi * O_SH, (oi + 1) * O_SH)
        wt_quads.append(np.ascontiguousarray(weight[os_].T))
        bt_quads.append(np.ascontiguousarray(B[os_].T))
        bias_quads.append(np.ascontiguousarray(bias[os_]))

    nc = _get_nc()
    in_maps = []
    for c in range(N_CORES):
        mi, oi = divmod(c, O_SPLIT)
        in_maps.append(
            {
                "xt": xt_halves[mi],
                "wt": wt_quads[oi],
                "bt": bt_quads[oi],
                "bias": bias_quads[oi],
                "a": A,
            }
        )

    res = run_bass_kernel_spmd(nc, in_maps, list(range(N_CORES)))
    LAST_RESULT = res

    out = np.empty((M, OUT_F), np.float32)
    for c in range(N_CORES):
        mi, oi = divmod(c, O_SPLIT)
        out[mi * M_SH : (mi + 1) * M_SH, oi * O_SH : (oi + 1) * O_SH] = (
            res.results[c]["out"]
        )
    return out.reshape(4, 2048, OUT_F)


# revision 19
# speedup vs baseline: 1.1645x; 1.0209x over previous
"""LoRALinear Trainium2 kernel.

out = x @ W^T + bias + 2.0 * ((x @ A^T) @ B^T)

Strategy:
  - 2D sharding over 8 NeuronCores: M (token) dim split 2-way, out_features
    split 4-way. Each core: x-shard [4096, 4096], W-shard [1024, 4096].
  - Host ships k-major (pre-transposed) layouts of x, W and B — pure data
    layout, all arithmetic stays on device.
  - LoRA is folded into the cached weights on-chip: WeffT = W^T + (2*B@A)^T,
    via 16-partition matmuls (A is naturally [r, k] so it is the stationary
    operand with no transpose needed).
  - fp16 compute: f32->f16 cast during DMA (SWDGE), fp16 matmuls (1 cycle/row)
    accumulating in fp32 PSUM, bias added on PSUM->SBUF evict (DVE).
  - Phase 0 (W streaming) keeps the PE dense: 3 m-blocks k-outer-interleaved
    with the fold (6 PSUM banks + 2 fold banks), a 12-deep W staging pool so
    the W stream never waits on compute, and small filler matmuls per k-tile
    so the HAM clock gate stays at 2.4 GHz throughout.
"""

import numpy as np

IN_F = 4096
OUT_F = 4096
R = 16
SCALING = 2.0
M = 4 * 2048  # 8192 tokens

N_CORES = 8
M_SPLIT = 2
O_SPLIT = 4
M_SH = M // M_SPLIT      # 4096 rows per core
O_SH = OUT_F // O_SPLIT  # 1024 out-features per core
K = IN_F
KT = K // 128            # 32 k-tiles
MG = M_SH // 512         # 8 m-groups of 512 rows
NPH0 = 3                 # m-blocks interleaved into the W-stream phase

_NC_CACHE = {}
LAST_RESULT = None


def _build():
    import concourse.mybir as mybir
    import concourse.tile as tile
    from concourse import bacc

    f32, f16 = mybir.dt.float32, mybir.dt.float16

    nc = bacc.Bacc(
        "TRN2", target_bir_lowering=False, debug=False, num_devices=N_CORES
    )
    xt_d = nc.dram_tensor("xt", [K, M_SH], f32, kind="ExternalInput")
    wt_d = nc.dram_tensor("wt", [K, O_SH], f32, kind="ExternalInput")
    bt_d = nc.dram_tensor("bt", [R, O_SH], f32, kind="ExternalInput")
    bias_d = nc.dram_tensor("bias", [O_SH], f32, kind="ExternalInput")
    a_d = nc.dram_tensor("a", [R, K], f32, kind="ExternalInput")
    out_d = nc.dram_tensor("out", [M_SH, O_SH], f32, kind="ExternalOutput")

    with tile.TileContext(nc) as tc:
        with (
            tc.tile_pool(name="const", bufs=1) as const,
            tc.tile_pool(name="wtp", bufs=KT) as wtp,
            tc.tile_pool(name="wraw", bufs=8) as wraw,
            tc.tile_pool(name="xtp", bufs=KT + 16) as xtp,
            tc.tile_pool(name="outp", bufs=4) as outp,
        ):
            wt_tiles = [None] * KT
            with (
                tc.tile_pool(name="psum_ba", bufs=2, space="PSUM") as psum_ba,
                tc.tile_pool(name="psum_m", bufs=2 * NPH0, space="PSUM") as psum_m,
            ):
                # Warmup burst first in DVE+PE program order: junk memset
                # as soon as the DVE comes up, then dependency-free matmuls
                # flip the PE HAM clock gate (1.2 -> 2.4 GHz) and bridge
                # until the first W/x tiles land.
                junk = const.tile([128, 512], f16)
                nc.vector.memset(junk[:], 0.0)
                pwarm = psum_ba.tile([128, 512], f32, name="ba")
                for _ in range(20):
                    nc.tensor.matmul(
                        pwarm[:], junk[:, 0:128], junk[:], start=True, stop=True
                    )

                # ---- constants ----
                bias_bc = const.tile([128, O_SH], f32)
                a_sb = const.tile([R, K], f16)
                nc.gpsimd.dma_start(a_sb[:], a_d[:])
                btr = const.tile([R, O_SH], f16)
                nc.gpsimd.dma_start(btr[:], bt_d[:])
                bt_sb = const.tile([R, O_SH], f16)  # 2 * B^T
                nc.vector.tensor_scalar_mul(bt_sb[:], btr[:], SCALING)
                # bias via SWDGE: keeps the sync HWDGE queue head free for
                # the W/x phase-0 stream (bias isn't needed until the first
                # evict, ~70us in).
                nc.gpsimd.dma_start(
                    bias_bc[:], bias_d[:].partition_broadcast(128)
                )

                def evict(g, mb, p0, p1):
                    osb = outp.tile([128, O_SH], f32, name="osb")
                    nc.vector.tensor_add(osb[:, 0:512], p0[:], bias_bc[:, 0:512])
                    nc.vector.tensor_add(
                        osb[:, 512:1024], p1[:], bias_bc[:, 512:1024]
                    )
                    nc.sync.dma_start(
                        out_d[g * 512 + mb * 128 : g * 512 + (mb + 1) * 128, :],
                        osb[:],
                    )

                def mblock_mms(pcol, xts, kt, mb):
                    st, sp = kt == 0, kt == KT - 1
                    ms = slice(mb * 128, (mb + 1) * 128)
                    nc.tensor.matmul(
                        pcol[0][:], xts[kt][:, ms], wt_tiles[kt][:, 0:512],
                        start=st, stop=sp,
                    )
                    nc.tensor.matmul(
                        pcol[1][:], xts[kt][:, ms], wt_tiles[kt][:, 512:1024],
                        start=st, stop=sp,
                    )

                # ---- Phase 0: W and x(g0) stream interleaved on the sync
                # HWDGE queue (strict FIFO -> lockstep arrival at HBM rate),
                # x(g0) staged f32 and cast to f16 on the DVE. The fold for
                # k+1 issues before the main matmuls for k so the PE queue
                # head never sits on a fresh semaphore wait. ----
                PRE = 4  # DMA pipeline depth ahead of the fold
                wt_raws = [None] * KT
                xs_tiles = [None] * KT
                xts0 = [None] * KT

                def wx_dma(kt):
                    ks = slice(kt * 128, (kt + 1) * 128)
                    w_raw = wraw.tile([128, O_SH], f32, name="w_raw")
                    nc.sync.dma_start(w_raw[:], wt_d[ks, :])
                    xs = wraw.tile([128, 512], f32, name="xs")
                    nc.sync.dma_start(xs[:], xt_d[ks, 0:512])
                    wt_raws[kt] = w_raw
                    xs_tiles[kt] = xs

                def xcast(kt):
                    # ScalarE (idle otherwise) does the f32->f16 cast so the
                    # DVE only carries the fold adds in phase 0.
                    x_t = xtp.tile([128, 512], f16, name="x_t")
                    nc.scalar.copy(x_t[:], xs_tiles[kt][:])
                    xts0[kt] = x_t

                def fold(kt):
                    ks = slice(kt * 128, (kt + 1) * 128)
                    # Filler MMs: absorb DMA/DVE waits so the HAM
                    # busy-window never sees an idle PE during the W
                    # stream. The first iterations bridge the slow early
                    # DMA completions (receipt latency on a cold queue),
                    # so they get a much larger filler budget.
                    nj = 14 if kt < 6 else 2
                    jt = psum_ba.tile([128, 512], f32, name="ba")
                    for j in range(nj):
                        nc.tensor.matmul(
                            jt[:, (j % 4) * 128 : (j % 4 + 1) * 128],
                            junk[:, 0:128], junk[:, 0:128],
                            start=True, stop=True,
                        )
                    pba = psum_ba.tile([128, 512], f32, name="ba")
                    pbb = psum_ba.tile([128, 512], f32, name="ba")
                    wt_t = wtp.tile([128, O_SH], f16, name="wt_t")
                    nc.tensor.matmul(
                        pba[:], a_sb[:, ks], bt_sb[:, 0:512],
                        start=True, stop=True,
                    )
                    nc.tensor.matmul(
                        pbb[:], a_sb[:, ks], bt_sb[:, 512:1024],
                        start=True, stop=True,
                    )
                    nc.vector.tensor_add(
                        wt_t[:, 0:512], pba[:], wt_raws[kt][:, 0:512]
                    )
                    nc.vector.tensor_add(
                        wt_t[:, 512:1024], pbb[:], wt_raws[kt][:, 512:1024]
                    )
                    wt_tiles[kt] = wt_t

                for kt in range(PRE):
                    wx_dma(kt)
                for kt in range(PRE):
                    xcast(kt)
                fold(0)
                pcols = [
                    [psum_m.tile([128, 512], f32, name="pm") for h in (0, 1)]
                    for _ in range(NPH0)
                ]
                for kt in range(KT):
                    if kt + PRE < KT:
                        wx_dma(kt + PRE)
                    if kt + 1 < KT:
                        fold(kt + 1)
                    if kt + PRE < KT:
                        xcast(kt + PRE)
                    for mb in range(NPH0):
                        mblock_mms(pcols[mb], xts0, kt, mb)
                for mb in range(NPH0):
                    evict(0, mb, pcols[mb][0], pcols[mb][1])

                # ---- remaining m-blocks: g0's tail, then groups 1..MG-1
                # with x streamed f32->f16 on the gpsimd SWDGE ----
                x_groups = [xts0]
                for g in range(1, MG):
                    gs = slice(g * 512, (g + 1) * 512)
                    xts = []
                    for kt in range(KT):
                        x_t = xtp.tile([128, 512], f16, name="x_t")
                        nc.gpsimd.dma_start(
                            x_t[:], xt_d[kt * 128 : (kt + 1) * 128, gs]
                        )
                        xts.append(x_t)
                    x_groups.append(xts)

                for g in range(MG):
                    xts = x_groups[g]
                    remaining = range(NPH0, 4) if g == 0 else range(4)
                    for mb in remaining:
                        p0 = psum_m.tile([128, 512], f32, name="pm")
                        p1 = psum_m.tile([128, 512], f32, name="pm")
                        for kt in range(KT):
                            mblock_mms((p0, p1), xts, kt, mb)
                        evict(g, mb, p0, p1)

    nc.compile()
    return nc


def _get_nc():
    if "nc" not in _NC_CACHE:
        _NC_CACHE["nc"] = _build()
    return _NC_CACHE["nc"]


def kernel(x, weight, bias, A, B):
    global LAST_RESULT
    from concourse.bass_utils import run_bass_kernel_spmd

    x = np.asarray(x, dtype=np.float32).reshape(M, K)
    weight = np.asarray(weight, dtype=np.float32)
    bias = np.asarray(bias, dtype=np.float32)
    A = np.ascontiguousarray(np.asarray(A, dtype=np.float32))
    B = np.asarray(B, dtype=np.float32)

    # Host-side layout prep (transposes only; no arithmetic).
    xt_halves = [
        np.ascontiguousarray(x[mi * M_SH : (mi + 1) * M_SH].T)
        for mi in range(M_SPLIT)
    ]
    wt_quads = []
    bt_quads = []
    bias_quads = []
    for oi in range(O_SPLIT):
        os_ = slice(oi * O_SH, (oi + 1) * O_SH)
        wt_quads.append(np.ascontiguousarray(weight[os_].T))
        bt_quads.append(np.ascontiguousarray(B[os_].T))
        bias_quads.append(np.ascontiguousarray(bias[os_]))

    nc = _get_nc()
    in_maps = []
    for c in range(N_CORES):
        mi, oi = divmod(c, O_SPLIT)
        in_maps.append(
            {
                "xt": xt_halves[mi],
                "wt": wt_quads[oi],
                "bt": bt_quads[oi],
                "bias": bias_quads[oi],
                "a": A,
            }
        )

    res = run_bass_kernel_spmd(nc, in_maps, list(range(N_CORES)))
    LAST_RESULT = res

    out = np.empty((M, OUT_F), np.float32)
    for c in range(N_CORES):
        mi, oi = divmod(c, O_SPLIT)
        out[mi * M_SH : (mi + 1) * M_SH, oi * O_SH : (oi + 1) * O_SH] = (
            res.results[c]["out"]
        )
    return out.reshape(4, 2048, OUT_F)


# revision 24
# speedup vs baseline: 1.1839x; 1.0167x over previous
"""LoRALinear Trainium2 kernel.

out = x @ W^T + bias + 2.0 * ((x @ A^T) @ B^T)

Strategy:
  - 2D sharding over 8 NeuronCores: M (token) dim split 2-way, out_features
    split 4-way. Each core: x-shard [4096, 4096], W-shard [1024, 4096].
  - Host ships k-major (pre-transposed) layouts of x, W and B — pure data
    layout, all arithmetic stays on device.
  - LoRA is folded into the cached weights on-chip: WeffT = W^T + (2*B@A)^T,
    via 16-partition matmuls (A is naturally [r, k] so it is the stationary
    operand with no transpose needed).
  - fp16 compute: f32->f16 cast during DMA (SWDGE), fp16 matmuls (1 cycle/row)
    accumulating in fp32 PSUM, bias added on PSUM->SBUF evict (DVE).
  - Phase 0 (W streaming) keeps the PE dense: 3 m-blocks k-outer-interleaved
    with the fold (6 PSUM banks + 2 fold banks), a 12-deep W staging pool so
    the W stream never waits on compute, and small filler matmuls per k-tile
    so the HAM clock gate stays at 2.4 GHz throughout.
"""

import numpy as np

IN_F = 4096
OUT_F = 4096
R = 16
SCALING = 2.0
M = 4 * 2048  # 8192 tokens

N_CORES = 8
M_SPLIT = 2
O_SPLIT = 4
M_SH = M // M_SPLIT      # 4096 rows per core
O_SH = OUT_F // O_SPLIT  # 1024 out-features per core
K = IN_F
KT = K // 128            # 32 k-tiles
MG = M_SH // 512         # 8 m-groups of 512 rows
NPH0 = 3                 # m-blocks interleaved into the W-stream phase

_NC_CACHE = {}
LAST_RESULT = None


def _build():
    import concourse.mybir as mybir
    import concourse.tile as tile
    from concourse import bacc

    f32, f16 = mybir.dt.float32, mybir.dt.float16

    nc = bacc.Bacc(
        "TRN2", target_bir_lowering=False, debug=False, num_devices=N_CORES
    )
    xt_d = nc.dram_tensor("xt", [K, M_SH], f32, kind="ExternalInput")
    wt_d = nc.dram_tensor("wt", [K, O_SH], f32, kind="ExternalInput")
    bt_d = nc.dram_tensor("bt", [R, O_SH], f32, kind="ExternalInput")
    bias_d = nc.dram_tensor("bias", [O_SH], f32, kind="ExternalInput")
    a_d = nc.dram_tensor("a", [R, K], f32, kind="ExternalInput")
    out_d = nc.dram_tensor("out", [M_SH, O_SH], f32, kind="ExternalOutput")

    with tile.TileContext(nc) as tc:
        with (
            tc.tile_pool(name="const", bufs=1) as const,
            tc.tile_pool(name="wtp", bufs=KT) as wtp,
            tc.tile_pool(name="wraw", bufs=12) as wraw,
            tc.tile_pool(name="xtp", bufs=KT + 16) as xtp,
            tc.tile_pool(name="outp", bufs=4) as outp,
        ):
            wt_tiles = [None] * KT
            with (
                tc.tile_pool(name="psum_ba", bufs=2, space="PSUM") as psum_ba,
                tc.tile_pool(name="psum_m", bufs=2 * NPH0, space="PSUM") as psum_m,
            ):
                # Warmup burst first in DVE+PE program order: junk memset
                # as soon as the DVE comes up, then dependency-free matmuls
                # flip the PE HAM clock gate (1.2 -> 2.4 GHz) and bridge
                # until the first W/x tiles land.
                junk = const.tile([128, 512], f16)
                nc.vector.memset(junk[:], 0.0)
                pwarm = psum_ba.tile([128, 512], f32, name="ba")
                for _ in range(20):
                    nc.tensor.matmul(
                        pwarm[:], junk[:, 0:128], junk[:], start=True, stop=True
                    )

                # ---- constants ----
                bias_bc = const.tile([128, O_SH], f32)
                a_sb = const.tile([R, K], f16)
                nc.gpsimd.dma_start(a_sb[:], a_d[:])
                btr = const.tile([R, O_SH], f16)
                nc.gpsimd.dma_start(btr[:], bt_d[:])
                bt_sb = const.tile([R, O_SH], f16)  # 2 * B^T
                nc.vector.tensor_scalar_mul(bt_sb[:], btr[:], SCALING)

                def evict(g, mb, p0, p1):
                    osb = outp.tile([128, O_SH], f32, name="osb")
                    nc.vector.tensor_add(osb[:, 0:512], p0[:], bias_bc[:, 0:512])
                    nc.vector.tensor_add(
                        osb[:, 512:1024], p1[:], bias_bc[:, 512:1024]
                    )
                    nc.sync.dma_start(
                        out_d[g * 512 + mb * 128 : g * 512 + (mb + 1) * 128, :],
                        osb[:],
                    )

                def mblock_mms(pcol, xts, kt, mb):
                    st, sp = kt == 0, kt == KT - 1
                    ms = slice(mb * 128, (mb + 1) * 128)
                    nc.tensor.matmul(
                        pcol[0][:], xts[kt][:, ms], wt_tiles[kt][:, 0:512],
                        start=st, stop=sp,
                    )
                    nc.tensor.matmul(
                        pcol[1][:], xts[kt][:, ms], wt_tiles[kt][:, 512:1024],
                        start=st, stop=sp,
                    )

                # ---- Phase 0: W streams f32 on the sync HWDGE queue at
                # HBM rate (deep staging pool), x(g0) streams f32->f16 on
                # the gpsimd SWDGE. The fold for k+1 issues before the main
                # matmuls for k so the PE queue head never sits on a fresh
                # semaphore wait. ----
                wt_raws = [None] * KT
                xts0 = [None] * KT
                for kt in range(KT):
                    w_raw = wraw.tile([128, O_SH], f32, name="w_raw")
                    nc.sync.dma_start(
                        w_raw[:], wt_d[kt * 128 : (kt + 1) * 128, :]
                    )
                    wt_raws[kt] = w_raw
                # bias on sync AFTER the whole W stream (needed only at the
                # first evict, ~70us in; keeps the W stream head clean).
                nc.sync.dma_start(
                    bias_bc[:], bias_d[:].partition_broadcast(128)
                )
                for kt in range(KT):
                    x_t = xtp.tile([128, 512], f16, name="x_t")
                    nc.gpsimd.dma_start(
                        x_t[:], xt_d[kt * 128 : (kt + 1) * 128, 0:512]
                    )
                    xts0[kt] = x_t

                def fold(kt):
                    ks = slice(kt * 128, (kt + 1) * 128)
                    # Filler MMs: absorb DMA/DVE waits so the HAM
                    # busy-window never sees an idle PE during the W
                    # stream. The first iterations bridge the slow early
                    # DMA completions (receipt latency on a cold queue),
                    # so they get a much larger filler budget.
                    nj = 10 if kt < 6 else 2
                    jt = psum_ba.tile([128, 512], f32, name="ba")
                    for j in range(nj):
                        nc.tensor.matmul(
                            jt[:, (j % 4) * 128 : (j % 4 + 1) * 128],
                            junk[:, 0:128], junk[:, 0:128],
                            start=True, stop=True,
                        )
                    pba = psum_ba.tile([128, 512], f32, name="ba")
                    pbb = psum_ba.tile([128, 512], f32, name="ba")
                    wt_t = wtp.tile([128, O_SH], f16, name="wt_t")
                    nc.tensor.matmul(
                        pba[:], a_sb[:, ks], bt_sb[:, 0:512],
                        start=True, stop=True,
                    )
                    nc.tensor.matmul(
                        pbb[:], a_sb[:, ks], bt_sb[:, 512:1024],
                        start=True, stop=True,
                    )
                    nc.vector.tensor_add(
                        wt_t[:, 0:512], pba[:], wt_raws[kt][:, 0:512]
                    )
                    nc.vector.tensor_add(
                        wt_t[:, 512:1024], pbb[:], wt_raws[kt][:, 512:1024]
                    )
                    wt_tiles[kt] = wt_t

                fold(0)
                pcols = [
                    [psum_m.tile([128, 512], f32, name="pm") for h in (0, 1)]
                    for _ in range(NPH0)
                ]
                for kt in range(KT):
                    if kt + 1 < KT:
                        fold(kt + 1)
                    for mb in range(NPH0):
                        mblock_mms(pcols[mb], xts0, kt, mb)
                for mb in range(NPH0):
                    evict(0, mb, pcols[mb][0], pcols[mb][1])

                # ---- remaining m-blocks: g0's tail, then groups 1..MG-1
                # with x streamed f32->f16 on the gpsimd SWDGE ----
                x_groups = [xts0]
                for g in range(1, MG):
                    gs = slice(g * 512, (g + 1) * 512)
                    xts = []
                    for kt in range(KT):
                        x_t = xtp.tile([128, 512], f16, name="x_t")
                        nc.gpsimd.dma_start(
                            x_t[:], xt_d[kt * 128 : (kt + 1) * 128, gs]
                        )
                        xts.append(x_t)
                    x_groups.append(xts)

                for g in range(MG):
                    xts = x_groups[g]
                    remaining = range(NPH0, 4) if g == 0 else range(4)
                    for mb in remaining:
                        p0 = psum_m.tile([128, 512], f32, name="pm")
                        p1 = psum_m.tile([128, 512], f32, name="pm")
                        for kt in range(KT):
                            mblock_mms((p0, p1), xts, kt, mb)
                        evict(g, mb, p0, p1)

    nc.compile()
    return nc


def _get_nc():
    if "nc" not in _NC_CACHE:
        _NC_CACHE["nc"] = _build()
    return _NC_CACHE["nc"]


def kernel(x, weight, bias, A, B):
    global LAST_RESULT
    from concourse.bass_utils import run_bass_kernel_spmd

    x = np.asarray(x, dtype=np.float32).reshape(M, K)
    weight = np.asarray(weight, dtype=np.float32)
    bias = np.asarray(bias, dtype=np.float32)
    A = np.ascontiguousarray(np.asarray(A, dtype=np.float32))
    B = np.asarray(B, dtype=np.float32)

    # Host-side layout prep (transposes only; no arithmetic).
    xt_halves = [
        np.ascontiguousarray(x[mi * M_SH : (mi + 1) * M_SH].T)
        for mi in range(M_SPLIT)
    ]
    wt_quads = []
    bt_quads = []
    bias_quads = []
    for oi in range(O_SPLIT):
        os_ = slice(oi * O_SH, (oi + 1) * O_SH)
        wt_quads.append(np.ascontiguousarray(weight[os_].T))
        bt_quads.append(np.ascontiguousarray(B[os_].T))
        bias_quads.append(np.ascontiguousarray(bias[os_]))

    nc = _get_nc()
    in_maps = []
    for c in range(N_CORES):
        mi, oi = divmod(c, O_SPLIT)
        in_maps.append(
            {
                "xt": xt_halves[mi],
                "wt": wt_quads[oi],
                "bt": bt_quads[oi],
                "bias": bias_quads[oi],
                "a": A,
            }
        )

    res = run_bass_kernel_spmd(nc, in_maps, list(range(N_CORES)))
    LAST_RESULT = res

    out = np.empty((M, OUT_F), np.float32)
    for c in range(N_CORES):
        mi, oi = divmod(c, O_SPLIT)
        out[mi * M_SH : (mi + 1) * M_SH, oi * O_SH : (oi + 1) * O_SH] = (
            res.results[c]["out"]
        )
    return out.reshape(4, 2048, OUT_F)
